# revision 6
# baseline (speedup 1.0000x reference)
"""Trainium2 Bass kernel for nn_BatchResidualLstm.

Reference computation (T=1000, N=32, I=H=1024):
  1. SequenceWise BatchNorm over flattened (T*N, I), train-mode stats.
  2. gx = einsum(xn, W_ii) + (b_ii+b_ih+b_ic)            [T,N,3H]
  3. LSTM-ish recurrence over T:
       gates = gx_t + h @ W_ih.T + c @ W_ic.T            [N,3H]
       i,f,o = sigmoid(split(gates))
       cell  = tanh(h @ W_hh.T + b_hh)
       cy    = f*c + i*cell
       hy    = o*(tanh(cy) + xn_t)
  returns hy stacked over t -> [T,N,H]

Sharding: data-parallel over batch N across 8 cores (4 samples/core),
weights replicated (fp16, SBUF-resident), BN stats via one AllReduce.
On-chip layout is feature-major ([feature, time*batch]) everywhere;
PE transposes convert at the edges.
"""

import numpy as np

import concourse.bass as bass
import concourse.mybir as mybir
import concourse.tile as tile
from concourse.tile import ScopedClock, TileContext
from concourse.masks import make_identity

# ---------------------------------------------------------------------------
# Patch: this environment's walrus only accepts ONE sync-wait per sequencer
# CTRL instruction; Tile's kernel-tail drain aggregates many. Split them
# across chained nops on the sync engine.
# ---------------------------------------------------------------------------
def _split_drain_and_barrier(self, tick_clock, wait_clock):
    nc = self.nc
    probe = nc.sync.nop()
    wait_clock.add_sem_waits(probe.ins, ScopedClock({None: tick_clock.global_clock}))
    si = probe.ins.sync_info
    waits = list(si.on_wait) if si is not None else []
    if si is not None:
        probe.ins.sync_info = mybir.SyncInfo(on_wait=[], on_update=[])
    for w in waits:
        nop = nc.sync.nop()
        nop.ins.sync_info = mybir.SyncInfo(on_wait=[w], on_update=[])
    nc.sync.drain()
    nc.all_engine_barrier()
    popped = nc._tile_sem_poison_stack.pop()
    assert popped is self._sem_poison
    nc.clear_and_free_semaphores(list(self.sems.allocated().values()))
    nc.all_engine_barrier()


tile.TileContext._drain_and_barrier = _split_drain_and_barrier

_WAITNOP_ID = [0]


def _legalize_sync_waits(nc):
    """Hoist all-but-one sync waits of every instruction onto same-engine nops
    (this walrus encodes at most one wait per instruction)."""
    for f in nc.m.functions:
        for bb in f.blocks:
            il = bb.instructions
            new = []
            changed = False
            for ins in il:
                si = ins.sync_info
                if si is not None and len(si.on_wait) > 1:
                    changed = True
                    waits = list(si.on_wait)
                    for w in waits[:-1]:
                        nop = mybir.InstNoOp(
                            name=f"waitnop-{_WAITNOP_ID[0]}", ins=[], outs=[]
                        )
                        _WAITNOP_ID[0] += 1
                        nop.engine = ins.engine
                        nop.sync_info = mybir.SyncInfo(on_wait=[w], on_update=[])
                        new.append(nop)
                    ins.sync_info = mybir.SyncInfo(
                        on_wait=[waits[-1]], on_update=list(si.on_update)
                    )
                new.append(ins)
            if changed:
                il[:] = new


# ---------------------------------------------------------------------------

P = 128
N_FULL = 32
I_FEAT = 1024
H_FEAT = 1024
G3 = 3 * H_FEAT
BN_EPS = 1e-5
KI = I_FEAT // P      # 8 input-feature k-tiles
KH = H_FEAT // P      # 8 hidden-feature k-tiles
MG = G3 // P          # 24 gate m-tiles
FP16 = mybir.dt.float16
FP32 = mybir.dt.float32

STEPS_PER_ITER = 8    # recurrence steps unrolled per For_i iteration


def build_nc(T, n_cores, do_collective=True, unroll_static=False, skip_rec=False, rec_only=False):
    """Build the SPMD Bass program for one core (batch slice NB = 32/n_cores)."""
    NB = N_FULL // n_cores
    TB = T * NB                      # local flattened (t,b) count
    CB = STEPS_PER_ITER * NB         # tb columns per recurrence iteration
    n_iters = T // STEPS_PER_ITER
    assert T % STEPS_PER_ITER == 0

    nc = bass.Bass()

    # ---- I/O ----
    x_in = nc.dram_tensor("x", [T, NB, I_FEAT], FP32, kind="ExternalInput")
    gamma_in = nc.dram_tensor("gamma", [I_FEAT], FP32, kind="ExternalInput")
    beta_in = nc.dram_tensor("beta", [I_FEAT], FP32, kind="ExternalInput")
    w_ii_in = nc.dram_tensor("W_ii", [G3, I_FEAT], FP32, kind="ExternalInput")
    w_ih_in = nc.dram_tensor("W_ih", [G3, H_FEAT], FP32, kind="ExternalInput")
    w_ic_in = nc.dram_tensor("W_ic", [G3, H_FEAT], FP32, kind="ExternalInput")
    b_ii_in = nc.dram_tensor("b_ii", [G3], FP32, kind="ExternalInput")
    b_ih_in = nc.dram_tensor("b_ih", [G3], FP32, kind="ExternalInput")
    b_ic_in = nc.dram_tensor("b_ic", [G3], FP32, kind="ExternalInput")
    w_hh_in = nc.dram_tensor("W_hh", [H_FEAT, H_FEAT], FP32, kind="ExternalInput")
    b_hh_in = nc.dram_tensor("b_hh", [H_FEAT], FP32, kind="ExternalInput")
    y_out = nc.dram_tensor("y", [T, NB, H_FEAT], FP32, kind="ExternalOutput")

    x_flat = x_in.rearrange("t b i -> (t b) i")
    y_flat = y_out.rearrange("t b i -> (t b) i")

    # ---- DRAM scratch ----
    xT_dram = nc.dram_tensor("xT_scratch", [P, KI, TB], FP16)
    xnT_dram = nc.dram_tensor("xnT_scratch", [P, KI, TB], FP16)
    gxT_dram = nc.dram_tensor("gxT_scratch", [P, MG, TB], FP16)
    ysT_dram = nc.dram_tensor("ysT_scratch", [P, KH, TB], FP16)
    ar_in = nc.dram_tensor("ar_in", [P, 2 * KI], FP32)
    ar_out = nc.dram_tensor("ar_out", [P, 2 * KI], FP32, addr_space="Shared")

    # tb tiling for transposes
    n_tb_tiles = (TB + P - 1) // P

    with TileContext(nc) as tc:
        import contextlib

        ctx = contextlib.ExitStack()
        with ctx:
            singles = ctx.enter_context(tc.tile_pool(name="singles", bufs=1))
            ident32 = singles.tile([P, P], FP32)
            make_identity(nc, ident32)
            ident16 = singles.tile([P, P], FP16)
            make_identity(nc, ident16)

            # persistent per-feature vectors
            scale_sb = singles.tile([P, KI], FP32)     # gamma * rstd
            shift_sb = singles.tile([P, KI], FP32)     # beta - mean*scale
            bsum_sb = singles.tile([P, MG], FP32)      # b_ii+b_ih+b_ic, feature-major
            bhh_sb = singles.tile([P, KH], FP32)

            # ---------- Phase 1: x transpose to feature-major + BN stats ----------
            stats_sb = singles.tile([P, KI, n_tb_tiles, 6], FP32)
            if rec_only:
                nc.vector.memset(stats_sb.rearrange("p a b c -> p (a b c)"), 1.0)
            with (
                tc.tile_pool(name="ph1_in", bufs=3) as ph1_in,
                tc.tile_pool(name="ph1_ps", bufs=4, space="PSUM") as ph1_ps,
                tc.tile_pool(name="ph1_out", bufs=4) as ph1_out,
            ):
                for c in range(n_tb_tiles if not rec_only else 0):
                    r0 = c * P
                    rows = min(P, TB - r0)
                    xin = ph1_in.tile([P, I_FEAT], FP32)
                    nc.sync.dma_start(xin[:rows, :], x_flat[r0 : r0 + rows, :])
                    for ki in range(KI):
                        pst = ph1_ps.tile([P, P], FP32)
                        nc.tensor.transpose(
                            pst[:, :rows],
                            xin[:rows, ki * P : (ki + 1) * P],
                            ident32[:rows, :rows],
                        )
                        xt16 = ph1_out.tile([P, P], FP16)
                        nc.vector.tensor_copy(out=xt16[:, :rows], in_=pst[:, :rows])
                        nc.vector.bn_stats(
                            out=stats_sb[:, ki, c, :], in_=pst[:, :rows]
                        )
                        nc.sync.dma_start(
                            xT_dram[:, ki, r0 : r0 + rows], xt16[:, :rows]
                        )

            # ---------- Phase 1b: aggregate stats, AllReduce, affine params ----------
            with tc.tile_pool(name="stats", bufs=1) as sp:
                mv = sp.tile([P, KI, 2], FP32)  # local [mean, var] per feature
                for ki in range(KI):
                    nc.vector.bn_aggr(out=mv[:, ki, :], in_=stats_sb[:, ki, :, :])
                # pack [mean, var+mean^2]
                pack = sp.tile([P, 2 * KI], FP32)
                m2 = sp.tile([P, KI], FP32)
                nc.vector.tensor_mul(m2, mv[:, :, 0], mv[:, :, 0])
                nc.vector.tensor_add(pack[:, KI:], m2, mv[:, :, 1])
                nc.vector.tensor_copy(out=pack[:, :KI], in_=mv[:, :, 0])
                nc.sync.dma_start(ar_in[:, :], pack[:, :])
                if do_collective:
                    nc.gpsimd.collective_compute(
                        "AllReduce",
                        mybir.AluOpType.add,
                        replica_groups=[list(range(n_cores))],
                        ins=[ar_in.ap().opt()],
                        outs=[ar_out.ap().opt()],
                    )
                    red = ar_out
                else:
                    red = ar_in
                gsum = sp.tile([P, 2 * KI], FP32)
                nc.sync.dma_start(gsum[:, :], red[:, :])
                gmean = sp.tile([P, KI], FP32)
                gvar = sp.tile([P, KI], FP32)
                nc.scalar.mul(gmean, gsum[:, :KI], 1.0 / n_cores)
                nc.scalar.mul(gvar, gsum[:, KI:], 1.0 / n_cores)  # E[x^2] for now
                gm2 = sp.tile([P, KI], FP32)
                nc.vector.tensor_mul(gm2, gmean, gmean)
                nc.vector.tensor_sub(gvar, gvar, gm2)
                # rstd = 1/sqrt(var+eps)
                eps_t = sp.tile([P, 1], FP32)
                nc.vector.memset(eps_t, float(BN_EPS))
                rstd = sp.tile([P, KI], FP32)
                nc.scalar.activation(
                    out=rstd, in_=gvar,
                    func=mybir.ActivationFunctionType.Sqrt, bias=eps_t[:, 0:1],
                )
                nc.vector.reciprocal(rstd, rstd)
                # scale = gamma * rstd ; shift = beta - gmean*scale
                gam = sp.tile([P, KI], FP32)
                bet = sp.tile([P, KI], FP32)
                nc.sync.dma_start(gam[:, :], gamma_in.rearrange("(k p) -> p k", p=P))
                nc.sync.dma_start(bet[:, :], beta_in.rearrange("(k p) -> p k", p=P))
                nc.vector.tensor_mul(scale_sb, gam, rstd)
                t0 = sp.tile([P, KI], FP32)
                nc.vector.tensor_mul(t0, gmean, scale_sb)
                nc.vector.tensor_sub(shift_sb, bet, t0)
                # bias sums for gx, feature-major [P, MG]
                bi = sp.tile([P, MG], FP32)
                bh = sp.tile([P, MG], FP32)
                bc = sp.tile([P, MG], FP32)
                nc.sync.dma_start(bi[:, :], b_ii_in.rearrange("(m p) -> p m", p=P))
                nc.sync.dma_start(bh[:, :], b_ih_in.rearrange("(m p) -> p m", p=P))
                nc.sync.dma_start(bc[:, :], b_ic_in.rearrange("(m p) -> p m", p=P))
                nc.vector.tensor_add(bsum_sb, bi, bh)
                nc.vector.tensor_add(bsum_sb, bsum_sb, bc)
                nc.sync.dma_start(bhh_sb[:, :], b_hh_in.rearrange("(k p) -> p k", p=P))

            # ---------- Phase 2: W_ii transpose + big GEMM gx = W_ii @ xn ----------
            with (
                tc.tile_pool(name="wii_pool", bufs=1) as wii_pool,
                tc.tile_pool(name="ph2_w", bufs=3) as ph2_w,
                tc.tile_pool(name="ph2_ps", bufs=4, space="PSUM") as ph2_ps,
            ):
                wiiT = wii_pool.tile([P, KI, G3], FP16)  # [i_part, i_ktile, g]
                if rec_only:
                    nc.vector.memset(wiiT.rearrange("p a b -> p (a b)"), 0.0)
                for ki in range(KI if not rec_only else 0):
                    for m in range(MG):
                        wtmp = ph2_w.tile([P, P], FP32, tag="wtmp")
                        nc.sync.dma_start(
                            wtmp[:, :],
                            w_ii_in[m * P : (m + 1) * P, ki * P : (ki + 1) * P],
                        )
                        pst = ph2_ps.tile([P, P], FP32, tag="wps")
                        nc.tensor.transpose(pst, wtmp, ident32)
                        nc.vector.tensor_copy(
                            out=wiiT[:, ki, m * P : (m + 1) * P], in_=pst
                        )

                # GEMM over tb chunks of 512
                CH = 512
                n_ch = (TB + CH - 1) // CH
                with (
                    tc.tile_pool(name="ph2_x", bufs=2) as ph2_x,
                    tc.tile_pool(name="ph2_g", bufs=3) as ph2_g,
                ):
                    for c in range(n_ch if not rec_only else 0):
                        c0 = c * CH
                        cols = min(CH, TB - c0)
                        xn16 = ph2_x.tile([P, KI, CH], FP16, tag="xn16")
                        for ki in range(KI):
                            xt = ph2_x.tile([P, CH], FP16, tag="xt")
                            nc.sync.dma_start(
                                xt[:, :cols], xT_dram[:, ki, c0 : c0 + cols]
                            )
                            nc.scalar.activation(
                                out=xn16[:, ki, :cols], in_=xt[:, :cols],
                                func=mybir.ActivationFunctionType.Identity,
                                bias=shift_sb[:, ki : ki + 1],
                                scale=scale_sb[:, ki : ki + 1],
                            )
                            nc.sync.dma_start(
                                xnT_dram[:, ki, c0 : c0 + cols], xn16[:, ki, :cols]
                            )
                        for m in range(MG):
                            ps = ph2_ps.tile([P, CH], FP32, tag="gps")
                            for ki in range(KI):
                                nc.tensor.matmul(
                                    ps[:, :cols],
                                    wiiT[:, ki, m * P : (m + 1) * P],
                                    xn16[:, ki, :cols],
                                    start=(ki == 0),
                                    stop=(ki == KI - 1),
                                )
                            gx16 = ph2_g.tile([P, CH], FP16, tag="gx16")
                            nc.scalar.activation(
                                out=gx16[:, :cols], in_=ps[:, :cols],
                                func=mybir.ActivationFunctionType.Identity,
                                bias=bsum_sb[:, m : m + 1],
                            )
                            nc.sync.dma_start(
                                gxT_dram[:, m, c0 : c0 + cols], gx16[:, :cols]
                            )

            # ---------- Phase 3a: recurrent weight transposes ----------
            ctx3 = contextlib.ExitStack()
            ctx3.__enter__()
            wpool = ctx3.enter_context(tc.tile_pool(name="wrec", bufs=1))
            wihT = wpool.tile([P, KH, G3], FP16)
            wicT = wpool.tile([P, KH, G3], FP16)
            whhT = wpool.tile([P, KH, H_FEAT], FP16)
            with (
                tc.tile_pool(name="ph3a_w", bufs=3) as ph3a_w,
                tc.tile_pool(name="ph3a_ps", bufs=4, space="PSUM") as ph3a_ps,
            ):
                if rec_only:
                    for dst in (wihT, wicT, whhT):
                        nc.vector.memset(dst.rearrange("p a b -> p (a b)"), 0.0)
                for src, dst, mtiles in (
                    (w_ih_in, wihT, MG),
                    (w_ic_in, wicT, MG),
                    (w_hh_in, whhT, KH),
                ) if not rec_only else ():
                    for ki in range(KH):
                        for m in range(mtiles):
                            wtmp = ph3a_w.tile([P, P], FP32, tag="wtmp")
                            nc.sync.dma_start(
                                wtmp[:, :],
                                src[m * P : (m + 1) * P, ki * P : (ki + 1) * P],
                            )
                            pst = ph3a_ps.tile([P, P], FP32, tag="wps")
                            nc.tensor.transpose(pst, wtmp, ident32)
                            nc.vector.tensor_copy(
                                out=dst[:, ki, m * P : (m + 1) * P], in_=pst
                            )

            # ---------- Phase 3b: the recurrence ----------
            state = ctx3.enter_context(tc.tile_pool(name="state", bufs=1))
            h16 = [
                state.tile([P, KH, NB], FP16, name=f"h16_{i}", tag=f"h16_{i}")
                for i in range(2)
            ]
            c16 = [
                state.tile([P, KH, NB], FP16, name=f"c16_{i}", tag=f"c16_{i}")
                for i in range(2)
            ]
            c32 = [
                state.tile([P, KH, NB], FP32, name=f"c32_{i}", tag=f"c32_{i}")
                for i in range(2)
            ]
            for tbuf in (*h16, *c16, *c32):
                nc.vector.memset(tbuf[:], 0.0)

            rec_ps = ctx3.enter_context(tc.tile_pool(name="rec_ps", bufs=8, space="PSUM"))
            rec_io = ctx3.enter_context(tc.tile_pool(name="rec_io", bufs=2))
            rec_t = ctx3.enter_context(tc.tile_pool(name="rec_t", bufs=16))

            engs = (mybir.EngineType.PE, mybir.EngineType.DVE, mybir.EngineType.Activation)
            import contextlib as _cl

            def _iter_ctx():
                if skip_rec:
                    return _cl.nullcontext([])
                if unroll_static:
                    return _cl.nullcontext(list(range(0, TB, CB)))
                return tc.For_i(0, TB, CB, hint_engines=engs)

            if skip_rec:
                _skip_marker = True
            with _iter_ctx() as _ivs:
              for iv in (_ivs if (unroll_static or skip_rec) else [_ivs]):
                gx_ch = rec_io.tile([P, MG, CB], FP16, tag="gx_ch")
                xn_ch = rec_io.tile([P, KI, CB], FP16, tag="xn_ch")
                ys_ch = rec_io.tile([P, KH, CB], FP16, tag="ys_ch")
                nc.sync.dma_start(gx_ch[:], gxT_dram[:, :, bass.ds(iv, CB)])
                nc.sync.dma_start(xn_ch[:], xnT_dram[:, :, bass.ds(iv, CB)])

                for j in range(STEPS_PER_ITER):
                    jb = j * NB
                    pin = j % 2
                    pout = (j + 1) % 2
                    # h of previous step: from ys_ch for j>0, else persistent
                    if j == 0:
                        h_rd = h16[0]
                        h_sl = [h_rd[:, k, :] for k in range(KH)]
                    else:
                        h_sl = [ys_ch[:, k, jb - NB : jb] for k in range(KH)]
                    c16_rd = c16[pin]
                    c32_rd = c32[pin]

                    # ---- matmuls: psum tiles hold 2 feature-blocks x (i,f,o,cell)
                    # c-dependent (W_ic) matmuls are emitted first: c16 of the
                    # previous step is written before its h tail, so the PE can
                    # begin the next step while DVE/Act finish h.
                    # psum slots: [0..7] = ih gates + cell (as before),
                    # [8..15] = ic gate partials. Each slot's start..stop
                    # window is contiguous; ic windows complete first.
                    psums = [
                        rec_ps.tile([P, 16, NB], FP32, name=f"rec_q{q}", tag="rec")
                        for q in range(KH // 2)
                    ]
                    for q in range(KH // 2):
                        ps = psums[q]
                        for half in range(2):
                            fb = 2 * q + half
                            s = 4 * half
                            for gi, m in enumerate((fb, KH + fb, 2 * KH + fb)):
                                for k in range(KH):
                                    nc.tensor.matmul(
                                        ps[:, 8 + s + gi, :],
                                        wicT[:, k, m * P : (m + 1) * P],
                                        c16_rd[:, k, :],
                                        start=(k == 0),
                                        stop=(k == KH - 1),
                                    )
                    for q in range(KH // 2):
                        ps = psums[q]
                        for half in range(2):
                            fb = 2 * q + half
                            s = 4 * half
                            for gi, m in enumerate((fb, KH + fb, 2 * KH + fb)):
                                for k in range(KH):
                                    nc.tensor.matmul(
                                        ps[:, s + gi, :],
                                        wihT[:, k, m * P : (m + 1) * P],
                                        h_sl[k],
                                        start=(k == 0),
                                        stop=(k == KH - 1),
                                    )
                            for k in range(KH):
                                nc.tensor.matmul(
                                    ps[:, s + 3, :],
                                    whhT[:, k, fb * P : (fb + 1) * P],
                                    h_sl[k],
                                    start=(k == 0),
                                    stop=(k == KH - 1),
                                )

                    # ---- elementwise per feature block
                    for fb in range(KH):
                        q, half = fb // 2, fb % 2
                        ps = psums[q]
                        s = 4 * half
                        gates = rec_t.tile([P, 4, NB], FP32, tag="gates")
                        # gates[0:3] = sigmoid(psum_ic + gx + psum_ih); each add
                        # reads at most one PSUM operand (walrus constraint)
                        nc.vector.tensor_add(
                            gates[:, 0:3, :],
                            ps[:, 8 + s : 8 + s + 3, :],
                            gx_ch[:, fb :: KH, jb : jb + NB],
                        )
                        nc.vector.tensor_add(
                            gates[:, 0:3, :],
                            gates[:, 0:3, :],
                            ps[:, s : s + 3, :],
                        )
                        nc.scalar.activation(
                            out=gates[:, 0:3, :], in_=gates[:, 0:3, :],
                            func=mybir.ActivationFunctionType.Sigmoid,
                        )
                        # gates[3] = tanh(cell_psum + b_hh)
                        nc.scalar.activation(
                            out=gates[:, 3, :], in_=ps[:, s + 3, :],
                            func=mybir.ActivationFunctionType.Tanh,
                            bias=bhh_sb[:, fb : fb + 1],
                        )
                        tmp = rec_t.tile([P, 2, NB], FP32, tag="tmp")
                        nc.vector.tensor_mul(tmp[:, 0, :], gates[:, 1, :], c32_rd[:, fb, :])
                        nc.vector.tensor_mul(tmp[:, 1, :], gates[:, 0, :], gates[:, 3, :])
                        cy = c32[pout]
                        nc.vector.tensor_add(cy[:, fb, :], tmp[:, 0, :], tmp[:, 1, :])
                        nc.vector.tensor_copy(out=c16[pout][:, fb, :], in_=cy[:, fb, :])
                        th = rec_t.tile([P, NB], FP32, tag="th")
                        nc.scalar.activation(
                            out=th, in_=cy[:, fb, :],
                            func=mybir.ActivationFunctionType.Tanh,
                        )
                        nc.vector.tensor_add(th, th, xn_ch[:, fb, jb : jb + NB])
                        nc.vector.tensor_mul(
                            ys_ch[:, fb, jb : jb + NB], gates[:, 2, :], th
                        )
                        if j == STEPS_PER_ITER - 1:
                            nc.vector.tensor_copy(
                                out=h16[0][:, fb, :], in_=ys_ch[:, fb, jb : jb + NB]
                            )

                nc.sync.dma_start(ysT_dram[:, :, bass.ds(iv, CB)], ys_ch[:])

            ctx3.__exit__(None, None, None)

            # ---------- Phase 4: transpose ysT back to [T,NB,H] fp32 ----------
            with (
                tc.tile_pool(name="ph4_in", bufs=3) as ph4_in,
                tc.tile_pool(name="ph4_ps", bufs=4, space="PSUM") as ph4_ps,
                tc.tile_pool(name="ph4_out", bufs=3) as ph4_out,
            ):
                for c in range(n_tb_tiles if not rec_only else 1):
                    r0 = c * P
                    rows = min(P, TB - r0)
                    yout = ph4_out.tile([P, H_FEAT], FP32)
                    for ki in range(KH):
                        yt = ph4_in.tile([P, P], FP16)
                        nc.sync.dma_start(
                            yt[:, :rows], ysT_dram[:, ki, r0 : r0 + rows]
                        )
                        pst = ph4_ps.tile([P, P], FP16)
                        nc.tensor.transpose(pst[:, :], yt[:, :], ident16)
                        nc.vector.tensor_copy(
                            out=yout[:rows, ki * P : (ki + 1) * P],
                            in_=pst[:rows, :],
                        )
                    nc.sync.dma_start(y_flat[r0 : r0 + rows, :], yout[:rows, :])

    _legalize_sync_waits(nc)
    return nc


def _shard_inputs(inputs, T, n_cores):
    NB = N_FULL // n_cores
    full = {k: np.asarray(v) for k, v in inputs.items()}
    maps = []
    for c in range(n_cores):
        m = {
            "x": np.ascontiguousarray(full["x"][:, c * NB : (c + 1) * NB, :], np.float32),
            "gamma": full["gamma"].astype(np.float32),
            "beta": full["beta"].astype(np.float32),
            "W_ii": full["W_ii"].astype(np.float32),
            "W_ih": full["W_ih"].astype(np.float32),
            "W_ic": full["W_ic"].astype(np.float32),
            "b_ii": full["b_ii"].astype(np.float32),
            "b_ih": full["b_ih"].astype(np.float32),
            "b_ic": full["b_ic"].astype(np.float32),
            "W_hh": full["W_hh"].astype(np.float32),
            "b_hh": full["b_hh"].astype(np.float32),
        }
        maps.append(m)
    return maps


_NC_CACHE = {}


def run(inputs, T=1000, n_cores=8, trace=False, unroll_static=False):
    from concourse.bass_utils import run_bass_kernel_spmd

    key = (T, n_cores, unroll_static)
    if key not in _NC_CACHE:
        _NC_CACHE[key] = build_nc(
            T, n_cores, do_collective=True, unroll_static=unroll_static
        )
    nc = _NC_CACHE[key]
    in_maps = _shard_inputs(inputs, T, n_cores)
    res = run_bass_kernel_spmd(
        nc, in_maps, core_ids=list(range(n_cores)), trace=trace
    )
    ys = np.concatenate([r["y"] for r in res.results], axis=1)
    return ys, res


def kernel(**inputs) -> np.ndarray:
    ys, _ = run(inputs, T=1000, n_cores=8)
    return ys.astype(np.float32)

